# revision 3
# baseline (speedup 1.0000x reference)
"""Multi-head attention (B=4, T=2048, C=1024, H=16) on 8 trn2 NeuronCores.

Sharding: core c = 2*b + g handles batch b (of 4) and head-group g (of 2,
8 heads = 512 channels each). Each core computes q/k/v projections for its
512 channels, full TxT attention for its 8 heads, and the partial output
projection out_part = y_local @ Wo[:, g*512:(g+1)*512].T. Host sums the two
partials per batch and adds bo.

Mask trick: keys with mask!=0 contribute exactly 0 to softmax (exp(-inf)),
so the host compacts the key axis to the unmasked positions before the K/V
projections (~halves attention work). The compacted key count is padded to a
multiple of 128; padding lanes get a -1e30 bias fused into the exp.

On-chip layouts (per core):
  xT   [C=1024, T=2048]      x[b].T              (rhs of q-proj, lhsT of ...)
  xkT  [C=1024, TKP]         compacted x[b][keep].T
  qT   [512, 2048]           q.T  (head hl = partitions 64*hl..+64 of tile hl//2)
  kT   [512, TKP]            k.T  (same layout)
  vaug [TKP, 8*65]           v in natural layout, 65 cols per head: 64 data + ones
  S_T  [Tk tile 128, Tq]     scores transposed: softmax denom = partition sum,
                             obtained via the ones column of vaug (row 64 of y)
  yT   [512, 2048]           normalized attention output transposed
"""

import numpy as np
import ml_dtypes

import concourse.bass as bass
import concourse.mybir as mybir
import concourse.tile as tile
from concourse import bacc
from concourse.bass_utils import run_bass_kernel_spmd

F32 = mybir.dt.float32
BF16 = mybir.dt.bfloat16
NP_BF16 = ml_dtypes.bfloat16

B, T, C = 4, 2048, 1024
H, D = 16, 64
G = 2                 # head groups (cores per batch)
HL = H // G           # heads per core = 8
DL = HL * D           # local channels = 512
SCALE = 1.0 / np.sqrt(D)
NEG = -1e30
N_CORES = 8

_nc_cache: dict = {}


def _build_nc(tkp: int):
    """Build + compile the SPMD Bass program for padded key count tkp."""
    ntk = tkp // 128          # key partition-tiles
    nkc = C // 128            # contraction tiles over C = 8
    nmq = DL // 128           # qT/kT partition-tiles = 4
    assert tkp % 128 == 0

    nc = bacc.Bacc(None)

    xT_d = nc.dram_tensor("xT", [C, T], BF16, kind="ExternalInput")
    xkT_d = nc.dram_tensor("xkT", [C, tkp], BF16, kind="ExternalInput")
    wqT_d = nc.dram_tensor("wqT", [C, DL], BF16, kind="ExternalInput")
    wkT_d = nc.dram_tensor("wkT", [C, DL], BF16, kind="ExternalInput")
    wvT_d = nc.dram_tensor("wvT", [C, DL], BF16, kind="ExternalInput")
    woT_d = nc.dram_tensor("woT", [DL, C], BF16, kind="ExternalInput")
    bqp_d = nc.dram_tensor("bqp", [128, nmq], F32, kind="ExternalInput")
    bkp_d = nc.dram_tensor("bkp", [128, nmq], F32, kind="ExternalInput")
    bvp_d = nc.dram_tensor("bvp", [128, DL], F32, kind="ExternalInput")
    mbp_d = nc.dram_tensor("mbp", [128, ntk], F32, kind="ExternalInput")
    out_d = nc.dram_tensor("out", [T, C], F32, kind="ExternalOutput")

    with tile.TileContext(nc) as tc:
        with (
            tc.tile_pool(name="persist", bufs=1) as pp,
            tc.tile_pool(name="work", bufs=4) as wp,
            tc.tile_pool(name="psum", bufs=2, space="PSUM") as psp,
        ):
            # ---- persistent SBUF tensors ----
            def persist(shape, dt, tag):
                return pp.tile(shape, dt, tag=tag, name=tag)

            xT_t = [persist([128, T], BF16, f"xT{k}") for k in range(nkc)]
            xkT_t = [persist([128, tkp], BF16, f"xkT{k}") for k in range(nkc)]
            wqT_t = [persist([128, DL], BF16, f"wqT{k}") for k in range(nkc)]
            wkT_t = [persist([128, DL], BF16, f"wkT{k}") for k in range(nkc)]
            wvT_t = [persist([128, DL], BF16, f"wvT{k}") for k in range(nkc)]
            woT_t = [persist([128, C], BF16, f"woT{k}") for k in range(nmq)]
            qT_t = [persist([128, T], BF16, f"qT{m}") for m in range(nmq)]
            kT_t = [persist([128, tkp], BF16, f"kT{m}") for m in range(nmq)]
            va_t = [persist([128, HL * (D + 1)], BF16, f"va{t}") for t in range(ntk)]
            yT_t = [persist([128, T], BF16, f"yT{m}") for m in range(nmq)]
            bqp_t = persist([128, nmq], F32, "bqp")
            bkp_t = persist([128, nmq], F32, "bkp")
            bvp_t = persist([128, DL], F32, "bvp")
            mbp_t = persist([128, ntk], F32, "mbp")
            ones_t = persist([1, D], F32, "ones")

            # ---- input DMAs ----
            for k in range(nkc):
                nc.sync.dma_start(out=wvT_t[k][:], in_=wvT_d[k * 128:(k + 1) * 128, :])
            for k in range(nkc):
                nc.sync.dma_start(out=xkT_t[k][:], in_=xkT_d[k * 128:(k + 1) * 128, :])
            for k in range(nkc):
                nc.sync.dma_start(out=wqT_t[k][:], in_=wqT_d[k * 128:(k + 1) * 128, :])
                nc.sync.dma_start(out=wkT_t[k][:], in_=wkT_d[k * 128:(k + 1) * 128, :])
            for k in range(nkc):
                nc.sync.dma_start(out=xT_t[k][:], in_=xT_d[k * 128:(k + 1) * 128, :])
            for m in range(nmq):
                nc.sync.dma_start(out=woT_t[m][:], in_=woT_d[m * 128:(m + 1) * 128, :])
            nc.sync.dma_start(out=bqp_t[:], in_=bqp_d[:])
            nc.sync.dma_start(out=bkp_t[:], in_=bkp_d[:])
            nc.sync.dma_start(out=bvp_t[:], in_=bvp_d[:])
            nc.sync.dma_start(out=mbp_t[:], in_=mbp_d[:])
            nc.vector.memset(ones_t[:], 1.0)

            # ---- v projection: v = xk @ WvT (+bv), into vaug strided layout
            for t in range(ntk):
                nc.vector.memset(va_t[t][:], 1.0)  # pre-fill ones columns
                ps = psp.tile([128, DL], F32, tag="s")
                for k in range(nkc):
                    nc.tensor.matmul(
                        ps[:],
                        lhsT=xkT_t[k][:, t * 128:(t + 1) * 128],
                        rhs=wvT_t[k][:],
                        start=(k == 0), stop=(k == nkc - 1),
                    )
                dst = va_t[t][:].rearrange("p (h e) -> p h e", e=D + 1)[:, :, 0:D]
                src = ps[:].rearrange("p (h e) -> p h e", e=D)
                bv3 = bvp_t[:].rearrange("p (h e) -> p h e", e=D)
                nc.vector.tensor_add(dst, src, bv3)

            # ---- q/k projections: qT = Wq_s @ x.T (+bq per-partition) ----
            for m in range(nmq):
                for n in range(T // 512):
                    ps = psp.tile([128, 512], F32, tag="s")
                    for k in range(nkc):
                        nc.tensor.matmul(
                            ps[:],
                            lhsT=wqT_t[k][:, m * 128:(m + 1) * 128],
                            rhs=xT_t[k][:, n * 512:(n + 1) * 512],
                            start=(k == 0), stop=(k == nkc - 1),
                        )
                    nc.vector.tensor_scalar_add(
                        qT_t[m][:, n * 512:(n + 1) * 512], ps[:], bqp_t[:, m:m + 1]
                    )
                kchunks = [(s, min(512, tkp - s)) for s in range(0, tkp, 512)]
                for (s0, cn) in kchunks:
                    ps = psp.tile([128, 512], F32, tag="s")
                    for k in range(nkc):
                        nc.tensor.matmul(
                            ps[:, 0:cn],
                            lhsT=wkT_t[k][:, m * 128:(m + 1) * 128],
                            rhs=xkT_t[k][:, s0:s0 + cn],
                            start=(k == 0), stop=(k == nkc - 1),
                        )
                    nc.vector.tensor_scalar_add(
                        kT_t[m][:, s0:s0 + cn], ps[:, 0:cn], bkp_t[:, m:m + 1]
                    )

            # ---- attention ----
            EXPF = mybir.ActivationFunctionType.Exp
            for hl in range(HL):
                mrow, poff = hl // 2, 64 * (hl % 2)
                for tqc in range(2):
                    q0 = tqc * 1024
                    y_ps = psp.tile([D + 1, 1024], F32, tag="y")
                    for t in range(ntk):
                        s_ps = psp.tile([128, 1024], F32, tag="s")
                        for h2 in range(2):
                            nc.tensor.matmul(
                                s_ps[:, h2 * 512:(h2 + 1) * 512],
                                lhsT=kT_t[mrow][poff:poff + 64, t * 128:(t + 1) * 128],
                                rhs=qT_t[mrow][poff:poff + 64,
                                               q0 + h2 * 512:q0 + (h2 + 1) * 512],
                                start=True, stop=True,
                            )
                        p_sb = wp.tile([128, 1024], BF16, tag="p")
                        nc.scalar.activation(
                            p_sb[:], s_ps[:], EXPF,
                            bias=mbp_t[:, t:t + 1], scale=float(SCALE),
                        )
                        for h2 in range(2):
                            nc.tensor.matmul(
                                y_ps[:, h2 * 512:(h2 + 1) * 512],
                                lhsT=va_t[t][:, hl * (D + 1):(hl + 1) * (D + 1)],
                                rhs=p_sb[:, h2 * 512:(h2 + 1) * 512],
                                start=(t == 0), stop=(t == ntk - 1),
                            )
                    # normalize: y[:64] * (1/y[64]) broadcast over partitions
                    rec = wp.tile([1, 1024], F32, tag="rec")
                    nc.vector.reciprocal(rec[:], y_ps[D:D + 1, :])
                    bc_ps = psp.tile([D, 1024], F32, tag="y")
                    for h2 in range(2):
                        nc.tensor.matmul(
                            bc_ps[:, h2 * 512:(h2 + 1) * 512],
                            lhsT=ones_t[:],
                            rhs=rec[:, h2 * 512:(h2 + 1) * 512],
                            start=True, stop=True,
                        )
                    bc_sb = wp.tile([D, 1024], F32, tag="bc")
                    nc.vector.tensor_copy(bc_sb[:], bc_ps[:])
                    nc.vector.tensor_mul(
                        yT_t[mrow][poff:poff + 64, q0:q0 + 1024],
                        y_ps[0:D, :], bc_sb[:],
                    )

            # ---- output projection: out = y_local @ Wo_s.T ----
            for mt in range(T // 128):
                for n2 in range(C // 512):
                    o_ps = psp.tile([128, 512], F32, tag="s")
                    for kt in range(nmq):
                        nc.tensor.matmul(
                            o_ps[:],
                            lhsT=yT_t[kt][:, mt * 128:(mt + 1) * 128],
                            rhs=woT_t[kt][:, n2 * 512:(n2 + 1) * 512],
                            start=(kt == 0), stop=(kt == nmq - 1),
                        )
                    o_sb = wp.tile([128, 512], F32, tag="o")
                    nc.vector.tensor_copy(o_sb[:], o_ps[:])
                    nc.sync.dma_start(
                        out=out_d[mt * 128:(mt + 1) * 128, n2 * 512:(n2 + 1) * 512],
                        in_=o_sb[:],
                    )

    nc.compile()
    return nc


def _get_nc(tkp: int):
    if tkp not in _nc_cache:
        _nc_cache[tkp] = _build_nc(tkp)
    return _nc_cache[tkp]


def kernel(x, mask, Wk, bk, Wq, bq, Wv, bv, Wo, bo, _run_kwargs=None):
    x = np.asarray(x, dtype=np.float32)
    mask = np.asarray(mask)
    Wk, bk = np.asarray(Wk, np.float32), np.asarray(bk, np.float32)
    Wq, bq = np.asarray(Wq, np.float32), np.asarray(bq, np.float32)
    Wv, bv = np.asarray(Wv, np.float32), np.asarray(bv, np.float32)
    Wo, bo = np.asarray(Wo, np.float32), np.asarray(bo, np.float32)

    keep = [np.flatnonzero(mask[b] == 0) for b in range(B)]
    max_keep = max(len(kp) for kp in keep)
    tkp = max(128, -(-max_keep // 128) * 128)
    ntk = tkp // 128
    nmq = DL // 128

    nc = _get_nc(tkp)

    in_maps = []
    for b in range(B):
        xT = np.ascontiguousarray(x[b].T).astype(NP_BF16)
        xk = np.zeros((tkp, C), np.float32)
        xk[: len(keep[b])] = x[b][keep[b]]
        xkT = np.ascontiguousarray(xk.T).astype(NP_BF16)
        mb = np.zeros(tkp, np.float32)
        mb[len(keep[b]):] = NEG
        mbp = np.ascontiguousarray(mb.reshape(ntk, 128).T)
        for g in range(G):
            gs, ge = g * DL, (g + 1) * DL
            in_maps.append({
                "xT": xT,
                "xkT": xkT,
                "wqT": np.ascontiguousarray(Wq[gs:ge].T).astype(NP_BF16),
                "wkT": np.ascontiguousarray(Wk[gs:ge].T).astype(NP_BF16),
                "wvT": np.ascontiguousarray(Wv[gs:ge].T).astype(NP_BF16),
                "woT": np.ascontiguousarray(Wo[:, gs:ge].T).astype(NP_BF16),
                "bqp": np.ascontiguousarray(bq[gs:ge].reshape(nmq, 128).T),
                "bkp": np.ascontiguousarray(bk[gs:ge].reshape(nmq, 128).T),
                "bvp": np.ascontiguousarray(
                    np.broadcast_to(bv[gs:ge], (128, DL)).astype(np.float32)),
                "mbp": mbp,
            })

    kw = _run_kwargs or {}
    res = run_bass_kernel_spmd(nc, in_maps, list(range(N_CORES)), **kw)

    out = np.empty((B, T, C), np.float32)
    for b in range(B):
        out[b] = res.results[2 * b]["out"] + res.results[2 * b + 1]["out"] + bo
    if kw:
        kernel.last_result = res
    return out


# revision 7
# speedup vs baseline: 1.2276x; 1.2276x over previous
"""Multi-head attention (B=4, T=2048, C=1024, H=16) on 8 trn2 NeuronCores.

Sharding: core c = 2*b + g handles batch b (of 4) and head-group g (of 2,
8 heads = 512 channels each). Each core computes q/k/v projections for its
512 channels, full TxT attention for its 8 heads, and the partial output
projection out_part = y_local @ Wo[:, g*512:(g+1)*512].T. Host sums the two
partials per batch and adds bo.

Mask trick: keys with mask!=0 contribute exactly 0 to softmax (exp(-inf)),
so the host compacts the key axis to the unmasked positions before the K/V
projections (~halves attention work). The compacted key count is padded to a
multiple of 128; padding lanes get a -1e30 bias fused into the exp.

On-chip layouts (per core):
  xT   [C=1024, T=2048]      x[b].T              (rhs of q-proj, lhsT of ...)
  xkT  [C=1024, TKP]         compacted x[b][keep].T
  qT   [512, 2048]           q.T  (head hl = partitions 64*hl..+64 of tile hl//2)
  kT   [512, TKP]            k.T  (same layout)
  vaug [TKP, 8*65]           v in natural layout, 65 cols per head: 64 data + ones
  S_T  [Tk tile 128, Tq]     scores transposed: softmax denom = partition sum,
                             obtained via the ones column of vaug (row 64 of y)
  yT   [512, 2048]           normalized attention output transposed
"""

import numpy as np
import ml_dtypes

import concourse.bass as bass
import concourse.mybir as mybir
import concourse.tile as tile
from concourse import bacc
from concourse.bass_utils import run_bass_kernel_spmd

F32 = mybir.dt.float32
BF16 = mybir.dt.bfloat16
NP_BF16 = ml_dtypes.bfloat16

B, T, C = 4, 2048, 1024
H, D = 16, 64
G = 2                 # head groups (cores per batch)
HL = H // G           # heads per core = 8
DL = HL * D           # local channels = 512
SCALE = 1.0 / np.sqrt(D)
NEG = -1e30
N_CORES = 8

_nc_cache: dict = {}


def _build_nc(tkp: int):
    """Build + compile the SPMD Bass program for padded key count tkp."""
    ntk = tkp // 128          # key partition-tiles
    nkc = C // 128            # contraction tiles over C = 8
    nmq = DL // 128           # qT/kT partition-tiles = 4
    assert tkp % 128 == 0

    nc = bacc.Bacc(None)

    xT_d = nc.dram_tensor("xT", [C, T], BF16, kind="ExternalInput")
    xkT_d = nc.dram_tensor("xkT", [C, tkp], BF16, kind="ExternalInput")
    wqT_d = nc.dram_tensor("wqT", [C, DL], BF16, kind="ExternalInput")
    wkT_d = nc.dram_tensor("wkT", [C, DL], BF16, kind="ExternalInput")
    wvT_d = nc.dram_tensor("wvT", [C, DL], BF16, kind="ExternalInput")
    woT_d = nc.dram_tensor("woT", [DL, C], BF16, kind="ExternalInput")
    bqp_d = nc.dram_tensor("bqp", [128, nmq], F32, kind="ExternalInput")
    bkp_d = nc.dram_tensor("bkp", [128, nmq], F32, kind="ExternalInput")
    bvp_d = nc.dram_tensor("bvp", [128, DL], F32, kind="ExternalInput")
    mbp_d = nc.dram_tensor("mbp", [128, ntk], F32, kind="ExternalInput")
    out_d = nc.dram_tensor("out", [T, C], F32, kind="ExternalOutput")

    with tile.TileContext(nc) as tc:
        with (
            tc.tile_pool(name="persist", bufs=1) as pp,
            tc.tile_pool(name="work", bufs=4) as wp,
            tc.tile_pool(name="psum", bufs=2, space="PSUM") as psp,
        ):
            # ---- persistent SBUF tensors ----
            def persist(shape, dt, tag):
                return pp.tile(shape, dt, tag=tag, name=tag)

            xT_t = [persist([128, T], BF16, f"xT{k}") for k in range(nkc)]
            xkT_t = [persist([128, tkp], BF16, f"xkT{k}") for k in range(nkc)]
            wqT_t = [persist([128, DL], BF16, f"wqT{k}") for k in range(nkc)]
            wkT_t = [persist([128, DL], BF16, f"wkT{k}") for k in range(nkc)]
            wvT_t = [persist([128, DL], BF16, f"wvT{k}") for k in range(nkc)]
            woT_t = [persist([128, C], BF16, f"woT{k}") for k in range(nmq)]
            qT_t = [persist([128, T], BF16, f"qT{m}") for m in range(nmq)]
            kT_t = [persist([128, tkp], BF16, f"kT{m}") for m in range(nmq)]
            va_t = [persist([128, HL * (D + 1)], BF16, f"va{t}") for t in range(ntk)]
            yT_t = [persist([128, T], BF16, f"yT{m}") for m in range(nmq)]
            bqp_t = persist([128, nmq], F32, "bqp")
            bkp_t = persist([128, nmq], F32, "bkp")
            bvp_t = persist([128, DL], F32, "bvp")
            mbp_t = persist([128, ntk], F32, "mbp")
            ones_t = persist([1, D], F32, "ones")

            # ---- input DMAs ----
            for k in range(nkc):
                nc.sync.dma_start(out=wvT_t[k][:], in_=wvT_d[k * 128:(k + 1) * 128, :])
            for k in range(nkc):
                nc.sync.dma_start(out=xkT_t[k][:], in_=xkT_d[k * 128:(k + 1) * 128, :])
            for k in range(nkc):
                nc.sync.dma_start(out=wqT_t[k][:], in_=wqT_d[k * 128:(k + 1) * 128, :])
                nc.sync.dma_start(out=wkT_t[k][:], in_=wkT_d[k * 128:(k + 1) * 128, :])
            for k in range(nkc):
                nc.sync.dma_start(out=xT_t[k][:], in_=xT_d[k * 128:(k + 1) * 128, :])
            for m in range(nmq):
                nc.sync.dma_start(out=woT_t[m][:], in_=woT_d[m * 128:(m + 1) * 128, :])
            nc.sync.dma_start(out=bqp_t[:], in_=bqp_d[:])
            nc.sync.dma_start(out=bkp_t[:], in_=bkp_d[:])
            nc.sync.dma_start(out=bvp_t[:], in_=bvp_d[:])
            nc.sync.dma_start(out=mbp_t[:], in_=mbp_d[:])
            nc.vector.memset(ones_t[:], 1.0)

            # ---- v projection: v = xk @ WvT (+bv), into vaug strided layout
            for t in range(ntk):
                nc.vector.memset(va_t[t][:], 1.0)  # pre-fill ones columns
                ps = psp.tile([128, DL], F32, tag="s")
                for k in range(nkc):
                    nc.tensor.matmul(
                        ps[:],
                        lhsT=xkT_t[k][:, t * 128:(t + 1) * 128],
                        rhs=wvT_t[k][:],
                        start=(k == 0), stop=(k == nkc - 1),
                    )
                dst = va_t[t][:].rearrange("p (h e) -> p h e", e=D + 1)[:, :, 0:D]
                src = ps[:].rearrange("p (h e) -> p h e", e=D)
                bv3 = bvp_t[:].rearrange("p (h e) -> p h e", e=D)
                nc.vector.tensor_add(dst, src, bv3)

            # ---- q/k projections: qT = Wq_s @ x.T (+bq per-partition) ----
            for m in range(nmq):
                for n in range(T // 1024):
                    ps = psp.tile([128, 1024], F32, tag="s")
                    for h2 in range(2):
                        for k in range(nkc):
                            nc.tensor.matmul(
                                ps[:, h2 * 512:(h2 + 1) * 512],
                                lhsT=wqT_t[k][:, m * 128:(m + 1) * 128],
                                rhs=xT_t[k][:, n * 1024 + h2 * 512:
                                            n * 1024 + (h2 + 1) * 512],
                                start=(k == 0), stop=(k == nkc - 1),
                            )
                    nc.vector.tensor_scalar_add(
                        qT_t[m][:, n * 1024:(n + 1) * 1024], ps[:], bqp_t[:, m:m + 1]
                    )
                kchunks = [(s, min(1024, tkp - s)) for s in range(0, tkp, 1024)]
                for (s0, cn) in kchunks:
                    ps = psp.tile([128, 1024], F32, tag="s")
                    for (c0, cw) in [(o, min(512, cn - o)) for o in range(0, cn, 512)]:
                        for k in range(nkc):
                            nc.tensor.matmul(
                                ps[:, c0:c0 + cw],
                                lhsT=wkT_t[k][:, m * 128:(m + 1) * 128],
                                rhs=xkT_t[k][:, s0 + c0:s0 + c0 + cw],
                                start=(k == 0), stop=(k == nkc - 1),
                            )
                    nc.vector.tensor_scalar_add(
                        kT_t[m][:, s0:s0 + cn], ps[:, 0:cn], bkp_t[:, m:m + 1]
                    )

            # ---- attention ----
            EXPF = mybir.ActivationFunctionType.Exp
            for hl in range(HL):
                mrow, poff = hl // 2, 64 * (hl % 2)
                for tqc in range(2):
                    q0 = tqc * 1024
                    y_ps = psp.tile([D + 1, 1024], F32, tag="y")
                    for t in range(ntk):
                        s_ps = psp.tile([128, 1024], F32, tag="s")
                        for h2 in range(2):
                            nc.tensor.matmul(
                                s_ps[:, h2 * 512:(h2 + 1) * 512],
                                lhsT=kT_t[mrow][poff:poff + 64, t * 128:(t + 1) * 128],
                                rhs=qT_t[mrow][poff:poff + 64,
                                               q0 + h2 * 512:q0 + (h2 + 1) * 512],
                                start=True, stop=True,
                            )
                        p_sb = wp.tile([128, 1024], BF16, tag="p")
                        nc.scalar.activation(
                            p_sb[:], s_ps[:], EXPF,
                            bias=mbp_t[:, t:t + 1], scale=float(SCALE),
                        )
                        for h2 in range(2):
                            nc.tensor.matmul(
                                y_ps[:, h2 * 512:(h2 + 1) * 512],
                                lhsT=va_t[t][:, hl * (D + 1):(hl + 1) * (D + 1)],
                                rhs=p_sb[:, h2 * 512:(h2 + 1) * 512],
                                start=(t == 0), stop=(t == ntk - 1),
                            )
                    # normalize: y[:64] * (1/y[64]) broadcast over partitions
                    den = wp.tile([1, 1024], F32, tag="den")
                    nc.vector.tensor_copy(den[:], y_ps[D:D + 1, :])
                    rec = wp.tile([1, 1024], F32, tag="rec")
                    nc.vector.reciprocal_approx_fast(rec[:], den[:])
                    bc_ps = psp.tile([D, 1024], F32, tag="y")
                    for h2 in range(2):
                        nc.tensor.matmul(
                            bc_ps[:, h2 * 512:(h2 + 1) * 512],
                            lhsT=ones_t[:],
                            rhs=rec[:, h2 * 512:(h2 + 1) * 512],
                            start=True, stop=True,
                        )
                    bc_sb = wp.tile([D, 1024], F32, tag="bc")
                    nc.vector.tensor_copy(bc_sb[:], bc_ps[:])
                    nc.vector.tensor_mul(
                        yT_t[mrow][poff:poff + 64, q0:q0 + 1024],
                        y_ps[0:D, :], bc_sb[:],
                    )

            # ---- output projection: out = y_local @ Wo_s.T ----
            for mt in range(T // 128):
                o_ps = psp.tile([128, 1024], F32, tag="s")
                for h2 in range(2):
                    for kt in range(nmq):
                        nc.tensor.matmul(
                            o_ps[:, h2 * 512:(h2 + 1) * 512],
                            lhsT=yT_t[kt][:, mt * 128:(mt + 1) * 128],
                            rhs=woT_t[kt][:, h2 * 512:(h2 + 1) * 512],
                            start=(kt == 0), stop=(kt == nmq - 1),
                        )
                o_sb = wp.tile([128, 1024], F32, tag="o")
                nc.vector.tensor_copy(o_sb[:], o_ps[:])
                nc.sync.dma_start(
                    out=out_d[mt * 128:(mt + 1) * 128, :],
                    in_=o_sb[:],
                )

    nc.compile()
    return nc


def _get_nc(tkp: int):
    if tkp not in _nc_cache:
        _nc_cache[tkp] = _build_nc(tkp)
    return _nc_cache[tkp]


def kernel(x, mask, Wk, bk, Wq, bq, Wv, bv, Wo, bo, _run_kwargs=None):
    x = np.asarray(x, dtype=np.float32)
    mask = np.asarray(mask)
    Wk, bk = np.asarray(Wk, np.float32), np.asarray(bk, np.float32)
    Wq, bq = np.asarray(Wq, np.float32), np.asarray(bq, np.float32)
    Wv, bv = np.asarray(Wv, np.float32), np.asarray(bv, np.float32)
    Wo, bo = np.asarray(Wo, np.float32), np.asarray(bo, np.float32)

    keep = [np.flatnonzero(mask[b] == 0) for b in range(B)]
    max_keep = max(len(kp) for kp in keep)
    tkp = max(128, -(-max_keep // 128) * 128)
    ntk = tkp // 128
    nmq = DL // 128

    nc = _get_nc(tkp)

    in_maps = []
    for b in range(B):
        xT = np.ascontiguousarray(x[b].T).astype(NP_BF16)
        xk = np.zeros((tkp, C), np.float32)
        xk[: len(keep[b])] = x[b][keep[b]]
        xkT = np.ascontiguousarray(xk.T).astype(NP_BF16)
        mb = np.zeros(tkp, np.float32)
        mb[len(keep[b]):] = NEG
        mbp = np.ascontiguousarray(mb.reshape(ntk, 128).T)
        for g in range(G):
            gs, ge = g * DL, (g + 1) * DL
            in_maps.append({
                "xT": xT,
                "xkT": xkT,
                "wqT": np.ascontiguousarray(Wq[gs:ge].T).astype(NP_BF16),
                "wkT": np.ascontiguousarray(Wk[gs:ge].T).astype(NP_BF16),
                "wvT": np.ascontiguousarray(Wv[gs:ge].T).astype(NP_BF16),
                "woT": np.ascontiguousarray(Wo[:, gs:ge].T).astype(NP_BF16),
                "bqp": np.ascontiguousarray(bq[gs:ge].reshape(nmq, 128).T),
                "bkp": np.ascontiguousarray(bk[gs:ge].reshape(nmq, 128).T),
                "bvp": np.ascontiguousarray(
                    np.broadcast_to(bv[gs:ge], (128, DL)).astype(np.float32)),
                "mbp": mbp,
            })

    kw = _run_kwargs or {}
    res = run_bass_kernel_spmd(nc, in_maps, list(range(N_CORES)), **kw)

    out = np.empty((B, T, C), np.float32)
    for b in range(B):
        out[b] = res.results[2 * b]["out"] + res.results[2 * b + 1]["out"] + bo
    if kw:
        kernel.last_result = res
    return out


# revision 10
# speedup vs baseline: 1.2364x; 1.0071x over previous
"""Multi-head attention (B=4, T=2048, C=1024, H=16) on 8 trn2 NeuronCores.

Sharding: core c = 2*b + g handles batch b (of 4) and head-group g (of 2,
8 heads = 512 channels each). Each core computes q/k/v projections for its
512 channels, full TxT attention for its 8 heads, and the partial output
projection out_part = y_local @ Wo[:, g*512:(g+1)*512].T. Host sums the two
partials per batch and adds bo.

Mask trick: keys with mask!=0 contribute exactly 0 to softmax (exp(-inf)),
so the host compacts the key axis to the unmasked positions before the K/V
projections (~halves attention work). The compacted key count is padded to a
multiple of 128; padding lanes get a -1e30 bias fused into the exp.

On-chip layouts (per core):
  xT   [C=1024, T=2048]      x[b].T              (rhs of q-proj, lhsT of ...)
  xkT  [C=1024, TKP]         compacted x[b][keep].T
  qT   [512, 2048]           q.T  (head hl = partitions 64*hl..+64 of tile hl//2)
  kT   [512, TKP]            k.T  (same layout)
  vaug [TKP, 8*65]           v in natural layout, 65 cols per head: 64 data + ones
  S_T  [Tk tile 128, Tq]     scores transposed: softmax denom = partition sum,
                             obtained via the ones column of vaug (row 64 of y)
  yT   [512, 2048]           normalized attention output transposed
"""

import numpy as np
import ml_dtypes

import concourse.bass as bass
import concourse.mybir as mybir
import concourse.tile as tile
from concourse import bacc
from concourse.bass_utils import run_bass_kernel_spmd

F32 = mybir.dt.float32
BF16 = mybir.dt.bfloat16
NP_BF16 = ml_dtypes.bfloat16

B, T, C = 4, 2048, 1024
H, D = 16, 64
G = 2                 # head groups (cores per batch)
HL = H // G           # heads per core = 8
DL = HL * D           # local channels = 512
SCALE = 1.0 / np.sqrt(D)
NEG = -1e30
N_CORES = 8

_nc_cache: dict = {}


def _build_nc(tkp: int):
    """Build + compile the SPMD Bass program for padded key count tkp."""
    ntk = tkp // 128          # key partition-tiles
    nkc = C // 128            # contraction tiles over C = 8
    nmq = DL // 128           # qT/kT partition-tiles = 4
    assert tkp % 128 == 0

    nc = bacc.Bacc(None)

    xT_d = nc.dram_tensor("xT", [C, T], BF16, kind="ExternalInput")
    xkT_d = nc.dram_tensor("xkT", [C, tkp], BF16, kind="ExternalInput")
    wqT_d = nc.dram_tensor("wqT", [C, DL], BF16, kind="ExternalInput")
    wkT_d = nc.dram_tensor("wkT", [C, DL], BF16, kind="ExternalInput")
    wvT_d = nc.dram_tensor("wvT", [C, DL], BF16, kind="ExternalInput")
    woT_d = nc.dram_tensor("woT", [DL, C], BF16, kind="ExternalInput")
    bqp_d = nc.dram_tensor("bqp", [128, nmq], F32, kind="ExternalInput")
    bkp_d = nc.dram_tensor("bkp", [128, nmq], F32, kind="ExternalInput")
    bvp_d = nc.dram_tensor("bvp", [128, DL], F32, kind="ExternalInput")
    mbp_d = nc.dram_tensor("mbp", [128, ntk], F32, kind="ExternalInput")
    out_d = nc.dram_tensor("out", [T, C], F32, kind="ExternalOutput")

    with tile.TileContext(nc) as tc:
        with (
            tc.tile_pool(name="persist", bufs=1) as pp,
            tc.tile_pool(name="work", bufs=4) as wp,
            tc.tile_pool(name="psum", bufs=1, space="PSUM") as psp,
        ):
            # ---- persistent SBUF tensors ----
            def persist(shape, dt, tag):
                return pp.tile(shape, dt, tag=tag, name=tag)

            xT_t = [persist([128, T], BF16, f"xT{k}") for k in range(nkc)]
            xkT_t = [persist([128, tkp], BF16, f"xkT{k}") for k in range(nkc)]
            wqT_t = [persist([128, DL], BF16, f"wqT{k}") for k in range(nkc)]
            wkT_t = [persist([128, DL], BF16, f"wkT{k}") for k in range(nkc)]
            wvT_t = [persist([128, DL], BF16, f"wvT{k}") for k in range(nkc)]
            woT_t = [persist([128, C], BF16, f"woT{k}") for k in range(nmq)]
            qT_t = [persist([128, T], BF16, f"qT{m}") for m in range(nmq)]
            kT_t = [persist([128, tkp], BF16, f"kT{m}") for m in range(nmq)]
            va_t = [persist([128, HL * (D + 1)], BF16, f"va{t}") for t in range(ntk)]
            yT_t = [persist([128, T], BF16, f"yT{m}") for m in range(nmq)]
            bqp_t = persist([128, nmq], F32, "bqp")
            bkp_t = persist([128, nmq], F32, "bkp")
            bvp_t = persist([128, DL], F32, "bvp")
            mbp_t = persist([128, ntk], F32, "mbp")
            ones_t = persist([1, D], F32, "ones")

            # psum slots: "s" x2 (4 banks), "y" x1 (2 banks), "f" x1 (2 banks)
            def psum_tile(shape, tag, name):
                return psp.tile(shape, F32, tag=tag, name=name,
                                bufs=2 if tag == "s" else 1)

            # ---- input DMAs (order = priority) ----
            nc.sync.dma_start(out=bqp_t[:], in_=bqp_d[:])
            nc.sync.dma_start(out=bkp_t[:], in_=bkp_d[:])
            nc.sync.dma_start(out=bvp_t[:], in_=bvp_d[:])
            nc.sync.dma_start(out=mbp_t[:], in_=mbp_d[:])
            for k in range(nkc):
                nc.sync.dma_start(out=wvT_t[k][:], in_=wvT_d[k * 128:(k + 1) * 128, :])
            for k in range(nkc):
                nc.sync.dma_start(out=xkT_t[k][:], in_=xkT_d[k * 128:(k + 1) * 128, :])
            for k in range(nkc):
                nc.sync.dma_start(out=wqT_t[k][:], in_=wqT_d[k * 128:(k + 1) * 128, :])
                nc.sync.dma_start(out=wkT_t[k][:], in_=wkT_d[k * 128:(k + 1) * 128, :])
            for k in range(nkc):
                nc.sync.dma_start(out=xT_t[k][:], in_=xT_d[k * 128:(k + 1) * 128, :])
            for m in range(nmq):
                nc.sync.dma_start(out=woT_t[m][:], in_=woT_d[m * 128:(m + 1) * 128, :])
            nc.vector.memset(ones_t[:], 1.0)
            for t in range(ntk):
                nc.vector.memset(va_t[t][:], 1.0)  # ones columns of vaug

            uid = [0]

            # ---- emission units (chains of PE work + epilogue copy) ----
            def v_unit(t, tag):
                uid[0] += 1
                ps = psum_tile([128, DL], tag, f"vps{uid[0]}")
                for k in range(nkc):
                    nc.tensor.matmul(
                        ps[:],
                        lhsT=xkT_t[k][:, t * 128:(t + 1) * 128],
                        rhs=wvT_t[k][:],
                        start=(k == 0), stop=(k == nkc - 1),
                    )
                dst = va_t[t][:].rearrange("p (h e) -> p h e", e=D + 1)[:, :, 0:D]
                src = ps[:].rearrange("p (h e) -> p h e", e=D)
                bv3 = bvp_t[:].rearrange("p (h e) -> p h e", e=D)
                nc.vector.tensor_add(dst, src, bv3)

            def q_unit(m, n, tag):  # n: 512-chunk index of T
                uid[0] += 1
                ps = psum_tile([128, 512], tag, f"qps{uid[0]}")
                for k in range(nkc):
                    nc.tensor.matmul(
                        ps[:],
                        lhsT=wqT_t[k][:, m * 128:(m + 1) * 128],
                        rhs=xT_t[k][:, n * 512:(n + 1) * 512],
                        start=(k == 0), stop=(k == nkc - 1),
                    )
                nc.vector.tensor_scalar_add(
                    qT_t[m][:, n * 512:(n + 1) * 512], ps[:], bqp_t[:, m:m + 1]
                )

            def k_unit(m, s0, cn, tag):
                uid[0] += 1
                ps = psum_tile([128, 512], tag, f"kps{uid[0]}")
                for k in range(nkc):
                    nc.tensor.matmul(
                        ps[:, 0:cn],
                        lhsT=wkT_t[k][:, m * 128:(m + 1) * 128],
                        rhs=xkT_t[k][:, s0:s0 + cn],
                        start=(k == 0), stop=(k == nkc - 1),
                    )
                nc.vector.tensor_scalar_add(
                    kT_t[m][:, s0:s0 + cn], ps[:, 0:cn], bkp_t[:, m:m + 1]
                )

            def o_unit(mt, tag):
                uid[0] += 1
                ps = psum_tile([128, C], tag, f"ops{uid[0]}")
                for h2 in range(2):
                    for kt in range(nmq):
                        nc.tensor.matmul(
                            ps[:, h2 * 512:(h2 + 1) * 512],
                            lhsT=yT_t[kt][:, mt * 128:(mt + 1) * 128],
                            rhs=woT_t[kt][:, h2 * 512:(h2 + 1) * 512],
                            start=(kt == 0), stop=(kt == nmq - 1),
                        )
                o_sb = wp.tile([128, C], F32, tag="o", name=f"osb{uid[0]}", bufs=2)
                nc.vector.tensor_copy(o_sb[:], ps[:])
                nc.sync.dma_start(out=out_d[mt * 128:(mt + 1) * 128, :], in_=o_sb[:])

            def qk_units(m):
                us = [lambda tag, m=m, n=n: q_unit(m, n, tag) for n in range(T // 512)]
                for s0 in range(0, tkp, 512):
                    cn = min(512, tkp - s0)
                    us.append(lambda tag, m=m, s0=s0, cn=cn: k_unit(m, s0, cn, tag))
                return us

            # ---- startup: v proj + q/k for m=0, round-robin over s/s/f ----
            start_units = [lambda tag, t=t: v_unit(t, tag) for t in range(ntk)]
            start_units += qk_units(0)
            tags3 = ["s", "s", "f"]
            for i, u in enumerate(start_units):
                u(tags3[i % 3])

            # filler queue consumed inside the attention loop
            fillers = []
            for m in range(1, nmq):
                fillers += qk_units(m)

            EXPF = mybir.ActivationFunctionType.Exp
            FILL_EVERY = 2  # t-steps per filler unit

            def attention(hl, tqc):
                mrow, poff = hl // 2, 64 * (hl % 2)
                q0 = tqc * 1024
                uid[0] += 1
                y_ps = psum_tile([D + 1, 1024], "y", f"yps{uid[0]}")
                for t in range(ntk):
                    uid[0] += 1
                    s_ps = psum_tile([128, 1024], "s", f"sps{uid[0]}")
                    for h2 in range(2):
                        nc.tensor.matmul(
                            s_ps[:, h2 * 512:(h2 + 1) * 512],
                            lhsT=kT_t[mrow][poff:poff + 64, t * 128:(t + 1) * 128],
                            rhs=qT_t[mrow][poff:poff + 64,
                                           q0 + h2 * 512:q0 + (h2 + 1) * 512],
                            start=True, stop=True,
                        )
                    p_sb = wp.tile([128, 1024], BF16, tag="p", name=f"p{uid[0]}")
                    nc.scalar.activation(
                        p_sb[:], s_ps[:], EXPF,
                        bias=mbp_t[:, t:t + 1], scale=float(SCALE),
                    )
                    for h2 in range(2):
                        nc.tensor.matmul(
                            y_ps[:, h2 * 512:(h2 + 1) * 512],
                            lhsT=va_t[t][:, hl * (D + 1):(hl + 1) * (D + 1)],
                            rhs=p_sb[:, h2 * 512:(h2 + 1) * 512],
                            start=(t == 0), stop=(t == ntk - 1),
                        )
                    if t % FILL_EVERY == 1 and fillers:
                        fillers.pop(0)("f")
                # copy numerator+denominator off psum, then normalize from SBUF
                uid[0] += 1
                yu = wp.tile([D + 1, 1024], F32, tag="yu", name=f"yu{uid[0]}", bufs=2)
                nc.vector.tensor_copy(yu[:], y_ps[:])
                den = wp.tile([1, 1024], F32, tag="den", name=f"den{uid[0]}", bufs=2)
                nc.vector.tensor_copy(den[:], yu[D:D + 1, :])
                rec = wp.tile([1, 1024], F32, tag="rec", name=f"rec{uid[0]}", bufs=2)
                nc.vector.reciprocal_approx_fast(rec[:], den[:])
                bc_ps = psum_tile([D, 1024], "f", f"bc{uid[0]}")
                for h2 in range(2):
                    nc.tensor.matmul(
                        bc_ps[:, h2 * 512:(h2 + 1) * 512],
                        lhsT=ones_t[:],
                        rhs=rec[:, h2 * 512:(h2 + 1) * 512],
                        start=True, stop=True,
                    )
                bc_sb = wp.tile([D, 1024], F32, tag="bc", name=f"bcs{uid[0]}", bufs=2)
                nc.vector.tensor_copy(bc_sb[:], bc_ps[:])
                nc.vector.tensor_mul(
                    yT_t[mrow][poff:poff + 64, q0:q0 + 1024],
                    yu[0:D, :], bc_sb[:],
                )

            for tqc in range(2):
                if tqc == 1:
                    fillers.extend(
                        lambda tag, mt=mt: o_unit(mt, tag) for mt in range(T // 256)
                    )
                for hl in range(HL):
                    attention(hl, tqc)

            # remaining output-projection tiles
            for u in fillers:
                u("f")
            for mt in range(T // 256, T // 128):
                o_unit(mt, "s" if mt % 2 else "f")

    nc.compile()
    return nc


def _get_nc(tkp: int):
    if tkp not in _nc_cache:
        _nc_cache[tkp] = _build_nc(tkp)
    return _nc_cache[tkp]


def kernel(x, mask, Wk, bk, Wq, bq, Wv, bv, Wo, bo, _run_kwargs=None):
    x = np.asarray(x, dtype=np.float32)
    mask = np.asarray(mask)
    Wk, bk = np.asarray(Wk, np.float32), np.asarray(bk, np.float32)
    Wq, bq = np.asarray(Wq, np.float32), np.asarray(bq, np.float32)
    Wv, bv = np.asarray(Wv, np.float32), np.asarray(bv, np.float32)
    Wo, bo = np.asarray(Wo, np.float32), np.asarray(bo, np.float32)

    keep = [np.flatnonzero(mask[b] == 0) for b in range(B)]
    max_keep = max(len(kp) for kp in keep)
    tkp = max(128, -(-max_keep // 128) * 128)
    ntk = tkp // 128
    nmq = DL // 128

    nc = _get_nc(tkp)

    in_maps = []
    for b in range(B):
        xT = np.ascontiguousarray(x[b].T).astype(NP_BF16)
        xk = np.zeros((tkp, C), np.float32)
        xk[: len(keep[b])] = x[b][keep[b]]
        xkT = np.ascontiguousarray(xk.T).astype(NP_BF16)
        mb = np.zeros(tkp, np.float32)
        mb[len(keep[b]):] = NEG
        mbp = np.ascontiguousarray(mb.reshape(ntk, 128).T)
        for g in range(G):
            gs, ge = g * DL, (g + 1) * DL
            in_maps.append({
                "xT": xT,
                "xkT": xkT,
                "wqT": np.ascontiguousarray(Wq[gs:ge].T).astype(NP_BF16),
                "wkT": np.ascontiguousarray(Wk[gs:ge].T).astype(NP_BF16),
                "wvT": np.ascontiguousarray(Wv[gs:ge].T).astype(NP_BF16),
                "woT": np.ascontiguousarray(Wo[:, gs:ge].T).astype(NP_BF16),
                "bqp": np.ascontiguousarray(bq[gs:ge].reshape(nmq, 128).T),
                "bkp": np.ascontiguousarray(bk[gs:ge].reshape(nmq, 128).T),
                "bvp": np.ascontiguousarray(
                    np.broadcast_to(bv[gs:ge], (128, DL)).astype(np.float32)),
                "mbp": mbp,
            })

    kw = _run_kwargs or {}
    res = run_bass_kernel_spmd(nc, in_maps, list(range(N_CORES)), **kw)

    out = np.empty((B, T, C), np.float32)
    for b in range(B):
        out[b] = res.results[2 * b]["out"] + res.results[2 * b + 1]["out"] + bo
    if kw:
        kernel.last_result = res
    return out


# revision 11
# speedup vs baseline: 1.5106x; 1.2218x over previous
"""Multi-head attention (B=4, T=2048, C=1024, H=16) on 8 trn2 NeuronCores.

Sharding: core c = 2*b + g handles batch b (of 4) and head-group g (of 2,
8 heads = 512 channels each). Each core computes q/k/v projections for its
512 channels, full TxT attention for its 8 heads, and the partial output
projection out_part = y_local @ Wo[:, g*512:(g+1)*512].T. Host sums the two
partials per batch and adds bo.

Mask trick: keys with mask!=0 contribute exactly 0 to softmax (exp(-inf)),
so the host compacts the key axis to the unmasked positions before the K/V
projections (~halves attention work). The compacted key count is padded to a
multiple of 128; padding lanes get a -1e30 bias fused into the exp.

On-chip layouts (per core):
  xT   [C=1024, T=2048]      x[b].T              (rhs of q-proj, lhsT of ...)
  xkT  [C=1024, TKP]         compacted x[b][keep].T
  qT   [512, 2048]           q.T  (head hl = partitions 64*hl..+64 of tile hl//2)
  kT   [512, TKP]            k.T  (same layout)
  vaug [TKP, 8*65]           v in natural layout, 65 cols per head: 64 data + ones
  S_T  [Tk tile 128, Tq]     scores transposed: softmax denom = partition sum,
                             obtained via the ones column of vaug (row 64 of y)
  yT   [512, 2048]           normalized attention output transposed
"""

import numpy as np
import ml_dtypes

import concourse.bass as bass
import concourse.mybir as mybir
import concourse.tile as tile
from concourse import bacc
from concourse.bass_utils import run_bass_kernel_spmd

F32 = mybir.dt.float32
BF16 = mybir.dt.bfloat16
NP_BF16 = ml_dtypes.bfloat16

B, T, C = 4, 2048, 1024
H, D = 16, 64
G = 2                 # head groups (cores per batch)
HL = H // G           # heads per core = 8
DL = HL * D           # local channels = 512
SCALE = 1.0 / np.sqrt(D)
NEG = -1e30
N_CORES = 8

_nc_cache: dict = {}


def _build_nc(tkp: int):
    """Build + compile the SPMD Bass program for padded key count tkp."""
    ntk = tkp // 128          # key partition-tiles
    nkc = C // 128            # contraction tiles over C = 8
    nmq = DL // 128           # qT/kT partition-tiles = 4
    assert tkp % 128 == 0

    nc = bacc.Bacc(None)

    xT_d = nc.dram_tensor("xT", [C, T], BF16, kind="ExternalInput")
    xkT_d = nc.dram_tensor("xkT", [C, tkp], BF16, kind="ExternalInput")
    wqT_d = nc.dram_tensor("wqT", [C, DL], BF16, kind="ExternalInput")
    wkT_d = nc.dram_tensor("wkT", [C, DL], BF16, kind="ExternalInput")
    wvT_d = nc.dram_tensor("wvT", [C, DL], BF16, kind="ExternalInput")
    woT_d = nc.dram_tensor("woT", [DL, C], BF16, kind="ExternalInput")
    bqp_d = nc.dram_tensor("bqp", [128, nmq], F32, kind="ExternalInput")
    bkp_d = nc.dram_tensor("bkp", [128, nmq], F32, kind="ExternalInput")
    bvp_d = nc.dram_tensor("bvp", [128, DL], F32, kind="ExternalInput")
    mbp_d = nc.dram_tensor("mbp", [128, ntk], F32, kind="ExternalInput")
    out_d = nc.dram_tensor("out", [T, C], F32, kind="ExternalOutput")

    with tile.TileContext(nc) as tc:
        with (
            tc.tile_pool(name="persist", bufs=1) as pp,
            tc.tile_pool(name="work", bufs=4) as wp,
            tc.tile_pool(name="psum", bufs=1, space="PSUM") as psp,
        ):
            # ---- persistent SBUF tensors ----
            def persist(shape, dt, tag):
                return pp.tile(shape, dt, tag=tag, name=tag)

            xT_t = [persist([128, T], BF16, f"xT{k}") for k in range(nkc)]
            xkT_t = [persist([128, tkp], BF16, f"xkT{k}") for k in range(nkc)]
            wqT_t = [persist([128, DL], BF16, f"wqT{k}") for k in range(nkc)]
            wkT_t = [persist([128, DL], BF16, f"wkT{k}") for k in range(nkc)]
            wvT_t = [persist([128, DL], BF16, f"wvT{k}") for k in range(nkc)]
            woT_t = [persist([128, C], BF16, f"woT{k}") for k in range(nmq)]
            qp_t = [persist([128, T], BF16, f"qp{h}") for h in range(HL)]
            kT_t = [persist([128, tkp], BF16, f"kT{m}") for m in range(nmq)]
            va_t = [persist([128, HL * (D + 1)], BF16, f"va{t}") for t in range(ntk)]
            yT_t = [persist([128, T], BF16, f"yT{m}") for m in range(nmq)]
            bqp_t = persist([128, nmq], F32, "bqp")
            bkp_t = persist([128, nmq], F32, "bkp")
            bvp_t = persist([128, DL], F32, "bvp")
            mbp_t = persist([128, ntk], F32, "mbp")
            ones_t = persist([1, D], F32, "ones")

            # psum slots: "s" x2 (4 banks), "y" x1 (2 banks), "f" x1 (2 banks)
            def psum_tile(shape, tag, name):
                return psp.tile(shape, F32, tag=tag, name=name,
                                bufs=2 if tag == "s" else 1)

            # ---- input DMAs (order = priority) ----
            nc.sync.dma_start(out=bqp_t[:], in_=bqp_d[:])
            nc.sync.dma_start(out=bkp_t[:], in_=bkp_d[:])
            nc.sync.dma_start(out=bvp_t[:], in_=bvp_d[:])
            nc.sync.dma_start(out=mbp_t[:], in_=mbp_d[:])
            for k in range(nkc):
                nc.sync.dma_start(out=wvT_t[k][:], in_=wvT_d[k * 128:(k + 1) * 128, :])
            for k in range(nkc):
                nc.sync.dma_start(out=xkT_t[k][:], in_=xkT_d[k * 128:(k + 1) * 128, :])
            for k in range(nkc):
                nc.sync.dma_start(out=wqT_t[k][:], in_=wqT_d[k * 128:(k + 1) * 128, :])
                nc.sync.dma_start(out=wkT_t[k][:], in_=wkT_d[k * 128:(k + 1) * 128, :])
            for k in range(nkc):
                nc.sync.dma_start(out=xT_t[k][:], in_=xT_d[k * 128:(k + 1) * 128, :])
            for m in range(nmq):
                nc.sync.dma_start(out=woT_t[m][:], in_=woT_d[m * 128:(m + 1) * 128, :])
            nc.vector.memset(ones_t[:], 1.0)
            for t in range(ntk):
                nc.vector.memset(va_t[t][:], 1.0)  # ones columns of vaug
            for h in range(HL):
                z0 = 0 if h % 2 else 64   # zero the other head's rows
                nc.gpsimd.memset(qp_t[h][z0:z0 + 64, :], 0.0)

            uid = [0]

            # ---- emission units (chains of PE work + epilogue copy) ----
            def v_unit(t, tag):
                uid[0] += 1
                ps = psum_tile([128, DL], tag, f"vps{uid[0]}")
                for k in range(nkc):
                    nc.tensor.matmul(
                        ps[:],
                        lhsT=xkT_t[k][:, t * 128:(t + 1) * 128],
                        rhs=wvT_t[k][:],
                        start=(k == 0), stop=(k == nkc - 1),
                    )
                dst = va_t[t][:].rearrange("p (h e) -> p h e", e=D + 1)[:, :, 0:D]
                src = ps[:].rearrange("p (h e) -> p h e", e=D)
                bv3 = bvp_t[:].rearrange("p (h e) -> p h e", e=D)
                nc.vector.tensor_add(dst, src, bv3)

            def q_unit(m, n, tag):  # n: 512-chunk index of T
                uid[0] += 1
                ps = psum_tile([128, 512], tag, f"qps{uid[0]}")
                for k in range(nkc):
                    nc.tensor.matmul(
                        ps[:],
                        lhsT=wqT_t[k][:, m * 128:(m + 1) * 128],
                        rhs=xT_t[k][:, n * 512:(n + 1) * 512],
                        start=(k == 0), stop=(k == nkc - 1),
                    )
                for par in range(2):
                    h = 2 * m + par
                    r0 = 64 * par
                    nc.vector.tensor_scalar_add(
                        qp_t[h][r0:r0 + 64, n * 512:(n + 1) * 512],
                        ps[r0:r0 + 64, :], bqp_t[r0:r0 + 64, m:m + 1]
                    )

            def k_unit(m, s0, cn, tag):
                uid[0] += 1
                ps = psum_tile([128, 512], tag, f"kps{uid[0]}")
                for k in range(nkc):
                    nc.tensor.matmul(
                        ps[:, 0:cn],
                        lhsT=wkT_t[k][:, m * 128:(m + 1) * 128],
                        rhs=xkT_t[k][:, s0:s0 + cn],
                        start=(k == 0), stop=(k == nkc - 1),
                    )
                nc.vector.tensor_scalar_add(
                    kT_t[m][:, s0:s0 + cn], ps[:, 0:cn], bkp_t[:, m:m + 1]
                )

            def o_unit(mt, tag):
                uid[0] += 1
                ps = psum_tile([128, C], tag, f"ops{uid[0]}")
                for h2 in range(2):
                    for kt in range(nmq):
                        nc.tensor.matmul(
                            ps[:, h2 * 512:(h2 + 1) * 512],
                            lhsT=yT_t[kt][:, mt * 128:(mt + 1) * 128],
                            rhs=woT_t[kt][:, h2 * 512:(h2 + 1) * 512],
                            start=(kt == 0), stop=(kt == nmq - 1),
                        )
                o_sb = wp.tile([128, C], F32, tag="o", name=f"osb{uid[0]}", bufs=2)
                nc.vector.tensor_copy(o_sb[:], ps[:])
                nc.sync.dma_start(out=out_d[mt * 128:(mt + 1) * 128, :], in_=o_sb[:])

            def qk_units(m):
                us = [lambda tag, m=m, n=n: q_unit(m, n, tag) for n in range(T // 512)]
                for s0 in range(0, tkp, 512):
                    cn = min(512, tkp - s0)
                    us.append(lambda tag, m=m, s0=s0, cn=cn: k_unit(m, s0, cn, tag))
                return us

            # ---- startup: v proj + q/k for m=0, round-robin over s/s/f ----
            start_units = [lambda tag, t=t: v_unit(t, tag) for t in range(ntk)]
            start_units += qk_units(0)
            tags3 = ["s", "s", "f"]
            for i, u in enumerate(start_units):
                u(tags3[i % 3])

            # filler queue consumed inside the attention loop
            fillers = []
            for m in range(1, nmq):
                fillers += qk_units(m)

            EXPF = mybir.ActivationFunctionType.Exp
            FILL_EVERY = 2  # t-steps per filler unit

            def attention(hl, tqc):
                mrow, poff = hl // 2, 64 * (hl % 2)
                q0 = tqc * 1024
                uid[0] += 1
                y_ps = psum_tile([D + 1, 1024], "y", f"yps{uid[0]}")
                for t in range(ntk):
                    uid[0] += 1
                    s_ps = psum_tile([128, 1024], "s", f"sps{uid[0]}")
                    for h2 in range(2):
                        nc.tensor.matmul(
                            s_ps[:, h2 * 512:(h2 + 1) * 512],
                            lhsT=kT_t[mrow][:, t * 128:(t + 1) * 128],
                            rhs=qp_t[hl][:, q0 + h2 * 512:q0 + (h2 + 1) * 512],
                            start=True, stop=True,
                        )
                    p_sb = wp.tile([128, 1024], BF16, tag="p", name=f"p{uid[0]}")
                    nc.scalar.activation(
                        p_sb[:], s_ps[:], EXPF,
                        bias=mbp_t[:, t:t + 1], scale=float(SCALE),
                    )
                    for h2 in range(2):
                        nc.tensor.matmul(
                            y_ps[:, h2 * 512:(h2 + 1) * 512],
                            lhsT=va_t[t][:, hl * (D + 1):(hl + 1) * (D + 1)],
                            rhs=p_sb[:, h2 * 512:(h2 + 1) * 512],
                            start=(t == 0), stop=(t == ntk - 1),
                        )
                    if t % FILL_EVERY == 1 and fillers:
                        fillers.pop(0)("f")
                # copy numerator+denominator off psum, then normalize from SBUF
                uid[0] += 1
                yu = wp.tile([D + 1, 1024], F32, tag="yu", name=f"yu{uid[0]}", bufs=2)
                nc.vector.tensor_copy(yu[:], y_ps[:])
                den = wp.tile([1, 1024], F32, tag="den", name=f"den{uid[0]}", bufs=2)
                nc.vector.tensor_copy(den[:], yu[D:D + 1, :])
                rec = wp.tile([1, 1024], F32, tag="rec", name=f"rec{uid[0]}", bufs=2)
                nc.vector.reciprocal_approx_fast(rec[:], den[:])
                bc_ps = psum_tile([D, 1024], "f", f"bc{uid[0]}")
                for h2 in range(2):
                    nc.tensor.matmul(
                        bc_ps[:, h2 * 512:(h2 + 1) * 512],
                        lhsT=ones_t[:],
                        rhs=rec[:, h2 * 512:(h2 + 1) * 512],
                        start=True, stop=True,
                    )
                bc_sb = wp.tile([D, 1024], F32, tag="bc", name=f"bcs{uid[0]}", bufs=2)
                nc.vector.tensor_copy(bc_sb[:], bc_ps[:])
                nc.vector.tensor_mul(
                    yT_t[mrow][poff:poff + 64, q0:q0 + 1024],
                    yu[0:D, :], bc_sb[:],
                )

            for tqc in range(2):
                if tqc == 1:
                    fillers.extend(
                        lambda tag, mt=mt: o_unit(mt, tag) for mt in range(T // 256)
                    )
                for hl in range(HL):
                    attention(hl, tqc)

            # remaining output-projection tiles
            for u in fillers:
                u("f")
            for mt in range(T // 256, T // 128):
                o_unit(mt, "s" if mt % 2 else "f")

    nc.compile()
    return nc


def _get_nc(tkp: int):
    if tkp not in _nc_cache:
        _nc_cache[tkp] = _build_nc(tkp)
    return _nc_cache[tkp]


def kernel(x, mask, Wk, bk, Wq, bq, Wv, bv, Wo, bo, _run_kwargs=None):
    x = np.asarray(x, dtype=np.float32)
    mask = np.asarray(mask)
    Wk, bk = np.asarray(Wk, np.float32), np.asarray(bk, np.float32)
    Wq, bq = np.asarray(Wq, np.float32), np.asarray(bq, np.float32)
    Wv, bv = np.asarray(Wv, np.float32), np.asarray(bv, np.float32)
    Wo, bo = np.asarray(Wo, np.float32), np.asarray(bo, np.float32)

    keep = [np.flatnonzero(mask[b] == 0) for b in range(B)]
    max_keep = max(len(kp) for kp in keep)
    tkp = max(128, -(-max_keep // 128) * 128)
    ntk = tkp // 128
    nmq = DL // 128

    nc = _get_nc(tkp)

    in_maps = []
    for b in range(B):
        xT = np.ascontiguousarray(x[b].T).astype(NP_BF16)
        xk = np.zeros((tkp, C), np.float32)
        xk[: len(keep[b])] = x[b][keep[b]]
        xkT = np.ascontiguousarray(xk.T).astype(NP_BF16)
        mb = np.zeros(tkp, np.float32)
        mb[len(keep[b]):] = NEG
        mbp = np.ascontiguousarray(mb.reshape(ntk, 128).T)
        for g in range(G):
            gs, ge = g * DL, (g + 1) * DL
            in_maps.append({
                "xT": xT,
                "xkT": xkT,
                "wqT": np.ascontiguousarray(Wq[gs:ge].T).astype(NP_BF16),
                "wkT": np.ascontiguousarray(Wk[gs:ge].T).astype(NP_BF16),
                "wvT": np.ascontiguousarray(Wv[gs:ge].T).astype(NP_BF16),
                "woT": np.ascontiguousarray(Wo[:, gs:ge].T).astype(NP_BF16),
                "bqp": np.ascontiguousarray(bq[gs:ge].reshape(nmq, 128).T),
                "bkp": np.ascontiguousarray(bk[gs:ge].reshape(nmq, 128).T),
                "bvp": np.ascontiguousarray(
                    np.broadcast_to(bv[gs:ge], (128, DL)).astype(np.float32)),
                "mbp": mbp,
            })

    kw = _run_kwargs or {}
    res = run_bass_kernel_spmd(nc, in_maps, list(range(N_CORES)), **kw)

    out = np.empty((B, T, C), np.float32)
    for b in range(B):
        out[b] = res.results[2 * b]["out"] + res.results[2 * b + 1]["out"] + bo
    if kw:
        kernel.last_result = res
    return out
